# revision 20
# baseline (speedup 1.0000x reference)
"""Trainium2 Bass kernel for nn_Conv2d_selfAdapt (dense_cnn).

Math reduction (derived from the reference):
  The final einsum weight[(c*9+p), j] = KERN[p] is independent of output
  channel j, so all 512 output channels are identical:
      out[b, :, h, w] = S[b,h,w] - sum_p mask_p[b,h,w] * Sshift_p[b,h,w]
  where S = channel-sum of x, Sshift_p = zero-padded spatial shift of S,
  and mask = straight-through one-hot of argmax over the 8 gate channels
  (softmax is monotone, theta=1 -> argmax(LN(conv(x,w)) + gumbel)).

  The only heavy compute is the 3x3 conv (512 -> 8 ch).  It is computed as
  a 1x1 conv with M=81 outputs (9 taps x 8 ch + a ones-row giving S, taps
  duplicated) in fp32r (full PE rate), then the 9 per-tap partial maps are
  spatially shifted (zero-padded SBUF grid + shifted-window SBUF DMAs) and
  summed with a K=81 selection matmul (fp32r).  Per-pixel LN/gumbel/argmax/
  select runs on the vector engine in a pixel-partition layout obtained
  with 4 wide PE transposes per image (exact fp32 to protect the argmax).

  The device emits only the single [H*W] map per image; the host
  broadcasts it to the 512 identical output channels.

Sharding: pure data parallel, 2 images per core across 8 cores.
"""

import os
import sys

import numpy as np

for _p in ("/opt/trn_rl_repo", "/root/.axon_site/_ro/trn_rl_repo"):
    if os.path.isdir(_p) and _p not in sys.path:
        sys.path.insert(0, _p)

import concourse.bass as bass
import concourse.bacc as bacc
import concourse.tile as tile
from concourse import mybir
from contextlib import ExitStack

B, C, H, W = 16, 512, 48, 48
N_CORES = 8
BPC = B // N_CORES          # images per core
HW = H * W                  # 2304
G = W + 2                   # padded grid side (50)
EPS_LN = 1e-6
BIG = 1000.0
FP = mybir.dt.float32
FR = mybir.dt.float32r
# conv/sel chunks: 48-col row-aligned so PSUM copies stay rectangular
CCHUNKS = [(0, 10), (10, 10), (20, 10), (30, 10), (40, 8)]  # (row0, nrows)
# pack chunks: 128-px block aligned (4x512 + 256) for the transposes
PCHUNKS = [(0, 512), (512, 512), (1024, 512), (1536, 512), (2048, 256)]
NBLK = HW // 128            # 18 valid pixel blocks per image (20 padded)
# flat padded conv-partial layout: row pitch 49 (48 data + 1 zero spacer),
# data cell (r, w) at 50 + 49*r + w; head [0:50] and tail [2402:2452] zero.
GP = 49
GBASE = 50
GTOT = GBASE + GP * 48 + GP + 1  # 2452
PSHW = GP * 48               # 2352 (shifted window length per tap)

AL = mybir.AluOpType
AX = mybir.AxisListType


def build_nc():
    nc = bacc.Bacc("TRN2", target_bir_lowering=False, debug=False,
                   num_devices=N_CORES)

    x_d = nc.dram_tensor("x", [BPC, C, HW], FR, kind="ExternalInput")
    g_d = nc.dram_tensor("g", [BPC, 8, HW], FP, kind="ExternalInput")
    # merged constants: one FR pack (w73 | sel) + one FP pack
    # (iota | iotab | lnw | ident) so startup needs only two const DMAs
    cr_d = nc.dram_tensor("cpackr", [128, 341], FR, kind="ExternalInput")
    cf_d = nc.dram_tensor("cpackf", [128, 152], FP, kind="ExternalInput")
    out_d = nc.dram_tensor("out", [BPC, HW], FP, kind="ExternalOutput")

    with tile.TileContext(nc) as tc, ExitStack() as ctx:
        consts = ctx.enter_context(tc.tile_pool(name="consts", bufs=1))
        xpool = ctx.enter_context(tc.tile_pool(name="xp", bufs=1))
        work = ctx.enter_context(tc.tile_pool(name="work", bufs=2))
        vp = ctx.enter_context(tc.tile_pool(name="vp", bufs=2))
        psA = ctx.enter_context(tc.tile_pool(name="psA", bufs=3, space="PSUM"))
        psS = ctx.enter_context(tc.tile_pool(name="psS", bufs=2, space="PSUM"))
        psT = ctx.enter_context(tc.tile_pool(name="psT", bufs=1, space="PSUM"))
        psO = ctx.enter_context(tc.tile_pool(name="psO", bufs=1, space="PSUM"))

        cr = consts.tile([128, 341], FR, tag="cr")
        cf = consts.tile([128, 152], FP, tag="cf")
        eps_t = consts.tile([128, 1], FP, tag="eps")
        nc.vector.memset(eps_t, EPS_LN)
        nc.gpsimd.dma_start(out=cr, in_=cr_d[:])
        nc.gpsimd.dma_start(out=cf, in_=cf_d[:])
        w73 = cr[:, 0:324].rearrange("p (kt m) -> p kt m", m=81)
        sel = cr[0:81, 324:341]
        iota = cf[:, 0:8]
        iotab = cf[:, 8:16]
        lnw = cf[:, 16:24]
        ident = cf[:, 24:152]
        # warm the Sqrt activation table before the timed pipeline needs it
        warm_t = consts.tile([128, 1], FP, tag="warm")
        nc.scalar.activation(warm_t, eps_t, mybir.ActivationFunctionType.Sqrt,
                             bias=eps_t, scale=1.0)

        lnw_b = lnw.unsqueeze(1).broadcast_to([128, 20, 8])
        iota_b = iota.unsqueeze(1).broadcast_to([128, 20, 8])
        iotab_b = iotab.unsqueeze(1).broadcast_to([128, 20, 8])

        # ---- input loads (quarter-split for DMA/compute pipelining) --------
        # All of x streams on the sync HWDGE ring, image 0 fully before
        # image 1, so image 0's whole post-conv chain overlaps image 1's
        # input stream.  Shift DMAs use the scalar ring, everything small
        # uses the gpsimd software ring - nothing queues behind x.
        # xpool holds only one image's tiles (bufs=1 per kt tag): image
        # 1's loads
        # alias image 0's slots, so the pool's WAR dependency naturally
        # sequences image 1's input stream behind image 0's conv reads.
        xt = []
        for b in range(BPC):
            row = []
            for kt in range(4):
                t = xpool.tile([128, HW], FR, tag=f"x{kt}")
                row.append(t)
            xt.append(row)

        # packed layout: 5 row-groups of 25 rows (17 sel outputs + 8 gumbel);
        # row-group rg covers pixels [512*rg, 512*rg+512)
        packs = []
        for b in range(BPC):
            packed = work.tile([125, 512], FP, tag="packed")
            packs.append(packed)
            # pixel blocks 18,19 don't exist; zero them so the vector
            # pipeline sees benign values in the padded lanes (rows 96:100
            # are re-covered by the rg=3 gumbel DMA afterwards)
            nc.scalar.memzero(packed[96:125, 256:512])
            for rg, (off, ncol) in enumerate(PCHUNKS):
                nc.gpsimd.dma_start(
                    out=packed[25 * rg + 17:25 * rg + 25, 0:ncol],
                    in_=g_d[b, :, off:off + ncol])

        for b in range(BPC):
            for q in range(4):
                for kt in range(4):
                    nc.sync.dma_start(
                        out=xt[b][kt][:, q * 576:(q + 1) * 576],
                        in_=x_d[b, kt * 128:(kt + 1) * 128,
                                q * 576:(q + 1) * 576])

        for b in range(BPC):
            # ---- conv as 1x1 matmul in fp32r, row-aligned chunks ----------
            # gridF: flat zero-padded layout, row pitch 49 (spacer column
            # absorbs horizontal shift wrap; head/tail absorb vertical)
            gridF = work.tile([81, GTOT], FR, tag="grid")
            nc.scalar.memzero(gridF[:, 0:GBASE])
            nc.scalar.memzero(gridF[:, GBASE + GP * 48:GTOT])
            nc.scalar.memzero(
                gridF[:, GBASE:GBASE + GP * 48].rearrange(
                    "p (r w) -> p r w", w=GP)[:, :, 48:49])
            for grp in range(0, len(CCHUNKS), 2):
                pair = CCHUNKS[grp:grp + 2]
                Ps = []
                for _pi in range(len(pair)):
                    Pt = psA.tile([81, 480], FP, tag="A")
                    Ps.append(Pt)
                for kt in range(4):
                    # adjacent matmuls share the stationary weights
                    for P, (r0, nr) in zip(Ps, pair):
                        nc.tensor.matmul(
                            P[:, 0:nr * 48],
                            w73[:, kt, :],
                            xt[b][kt][:, r0 * 48:(r0 + nr) * 48],
                            start=(kt == 0),
                            stop=(kt == 3),
                        )
                for P, (r0, nr) in zip(Ps, pair):
                    nc.scalar.copy(
                        out=gridF[:, GBASE + GP * r0:GBASE + GP * (r0 + nr)]
                        .rearrange("p (r w) -> p r w", w=GP)[:, :, 0:48],
                        in_=P[:, 0:nr * 48].rearrange("p (h w) -> p h w", w=W),
                    )

            # ---- shifted per-tap rows: one contiguous run per partition --
            psh = work.tile([81, PSHW], FR, tag="psh")
            for t in range(9):
                ti, tj = divmod(t, 3)
                s = GBASE + GP * (ti - 1) + (tj - 1)
                nc.scalar.dma_start(
                    out=psh[t * 9:(t + 1) * 9, :],
                    in_=gridF[t * 9:(t + 1) * 9, s:s + PSHW],
                )
            pshv = psh.rearrange("p (r w) -> p r w", w=GP)

            # ---- selection matmul (fp32r): rows 0-7 logits, 8+t = Sp_t ----
            # compute engines can only start at partition 0/32/64/96, so
            # results go PSUM -> whole-image stage -> (DMA) packed row 25*rg
            packed = packs[b]
            stage = work.tile([17, HW], FP, tag="stage")
            for (r0, nr) in CCHUNKS:
                ncol = nr * 48
                Lc = psS.tile([17, 480], FP, tag="S")
                nc.tensor.matmul(Lc[:, 0:ncol], sel,
                                 pshv[:, r0:r0 + nr, 0:48],
                                 start=True, stop=True)
                nc.vector.tensor_copy(stage[:, r0 * 48:r0 * 48 + ncol],
                                      Lc[:, 0:ncol])
            for rg, (off, ncol) in enumerate(PCHUNKS):
                nc.gpsimd.dma_start(
                    out=packed[25 * rg:25 * rg + 17, 0:ncol],
                    in_=stage[:, off:off + ncol])

            # ---- transpose to pixel-partition layout (exact fp32) ---------
            # tt free layout [rg, cg, 25] so block index k = 4*rg+cg equals
            # the pixel-block index -> output rows come out in order
            tt = work.tile([128, 5, 4, 25], FP, tag="tt")
            for cg in range(4):
                tp = psT.tile([128, 125], FP, tag="T")
                nc.tensor.transpose(tp,
                                    packed[:, cg * 128:(cg + 1) * 128],
                                    ident[0:125, 0:125])
                nc.vector.tensor_copy(
                    tt[:, :, cg, :],
                    tp.rearrange("p (rg j) -> p rg j", j=25))
            ttv = tt.rearrange("p rg cg j -> p (rg cg) j")

            # ---- per-pixel LN + gumbel + argmax + neighbor select ---------
            Lap = ttv[:, :, 0:8]
            mu = vp.tile([128, 20], FP, tag="mu")
            nc.vector.tensor_reduce(mu, Lap, axis=AX.X, op=AL.add)
            cen = vp.tile([128, 20, 8], FP, tag="cen")
            nc.vector.scalar_tensor_tensor(
                cen, in0=mu.unsqueeze(2).broadcast_to([128, 20, 8]),
                scalar=-1.0 / 8.0, in1=Lap, op0=AL.mult, op1=AL.add)
            sq = vp.tile([128, 20, 8], FP, tag="sq")
            nc.vector.tensor_tensor(sq, cen, cen, op=AL.mult)
            v8 = vp.tile([128, 20], FP, tag="v8")
            nc.vector.tensor_reduce(v8, sq, axis=AX.X, op=AL.add)
            sd = vp.tile([128, 20], FP, tag="sd")
            nc.scalar.activation(sd, v8, mybir.ActivationFunctionType.Sqrt,
                                 bias=eps_t, scale=1.0 / 8.0)
            rstd = vp.tile([128, 20], FP, tag="rstd")
            nc.vector.reciprocal(rstd, sd)
            rl = vp.tile([128, 20, 8], FP, tag="rl")
            nc.vector.tensor_tensor(
                rl, rstd.unsqueeze(2).broadcast_to([128, 20, 8]), lnw_b,
                op=AL.mult)
            z = vp.tile([128, 20, 8], FP, tag="z")
            nc.vector.tensor_tensor(z, cen, rl, op=AL.mult)
            # add gumbel (+ln_bias, folded on host)
            z2 = vp.tile([128, 20, 8], FP, tag="z2")
            nc.vector.tensor_tensor(z2, z, ttv[:, :, 17:25], op=AL.add)

            mx = vp.tile([128, 20], FP, tag="mx")
            nc.vector.tensor_reduce(mx, z2, axis=AX.X, op=AL.max)
            eq = vp.tile([128, 20, 8], FP, tag="eq")
            nc.vector.tensor_tensor(
                eq, z2, mx.unsqueeze(2).broadcast_to([128, 20, 8]),
                op=AL.is_equal)
            im = vp.tile([128, 20, 8], FP, tag="im")
            nc.vector.scalar_tensor_tensor(
                im, in0=eq, scalar=-BIG, in1=iotab_b, op0=AL.mult, op1=AL.add)
            am = vp.tile([128, 20], FP, tag="am")
            nc.vector.tensor_reduce(am, im, axis=AX.X, op=AL.min)
            hard = vp.tile([128, 20, 8], FP, tag="hard")
            nc.vector.tensor_tensor(
                hard, iota_b, am.unsqueeze(2).broadcast_to([128, 20, 8]),
                op=AL.is_equal)

            # pair one-hot lanes with the 8 non-center taps (skip center=12)
            prod = vp.tile([128, 20, 8], FP, tag="prod")
            nc.vector.tensor_tensor(prod[:, :, 0:4], hard[:, :, 0:4],
                                    ttv[:, :, 8:12], op=AL.mult)
            nc.vector.tensor_tensor(prod[:, :, 4:8], hard[:, :, 4:8],
                                    ttv[:, :, 13:17], op=AL.mult)
            selS = vp.tile([128, 20], FP, tag="selS")
            nc.vector.tensor_reduce(selS, prod, axis=AX.X, op=AL.add)
            outm = vp.tile([128, 20], FP, tag="outm")
            nc.vector.tensor_tensor(outm, ttv[:, :, 12], selS,
                                    op=AL.subtract)

            # ---- transpose map back to pixel order and store --------------
            po = psO.tile([20, 128], FP, tag="O")
            nc.tensor.transpose(po, outm, ident)
            outsb = vp.tile([20, 128], FP, tag="outsb")
            nc.vector.tensor_copy(outsb, po)
            nc.gpsimd.dma_start(
                out=out_d[b, :].rearrange("(j f) -> j f", f=128),
                in_=outsb[0:18, :])

    nc.compile()
    return nc


def _to_fp32r(a):
    """Round fp32 -> fp32r (11-bit mantissa, low 12 bits zero, RNE).

    The PE's fp32r datapath reads only the top 20 bits; pre-rounding on
    the host lets the kernel DMA the data straight into float32r tiles.
    """
    u = np.ascontiguousarray(a, dtype=np.float32).view(np.uint32)
    r = (u + 0x7FF + ((u >> 12) & 1)) & np.uint32(0xFFFFF000)
    return r.view(np.float32)


def host_inputs(x, mask_weight, ln_weight, ln_bias, gumbel_noise):
    """Build per-core input maps (numpy only)."""
    x = _to_fp32r(np.asarray(x, dtype=np.float32)).reshape(B, C, HW)
    g = np.asarray(gumbel_noise, dtype=np.float32).reshape(B, 8, HW)
    # fold the LN bias into the (precomputed) gumbel noise
    g = np.ascontiguousarray(
        g + np.asarray(ln_bias, np.float32).reshape(1, 8, 1))

    mw = np.asarray(mask_weight, dtype=np.float32).reshape(8, C, 9)
    a = mw.transpose(1, 2, 0)                         # [c, tap, o]
    w73 = np.ones((C, 9, 9), dtype=np.float32)        # [c, tap, o|ones]
    w73[:, :, :8] = a
    w73 = w73.reshape(4, 128, 81).transpose(1, 0, 2)  # [c_mod, kt, m]
    w73 = _to_fp32r(w73)

    sel = np.zeros((81, 17), dtype=np.float32)
    for t in range(9):
        for o in range(8):
            sel[t * 9 + o, o] = 1.0
        sel[t * 9 + 8, 8 + t] = 1.0
    cpackr = np.zeros((128, 341), dtype=np.float32)
    cpackr[:, 0:324] = w73.reshape(128, 324)
    cpackr[0:81, 324:341] = sel
    cpackf = np.zeros((128, 152), dtype=np.float32)
    cpackf[:, 0:8] = np.arange(8, dtype=np.float32)
    cpackf[:, 8:16] = np.arange(8, dtype=np.float32) + BIG
    cpackf[:, 16:24] = np.asarray(ln_weight, np.float32).reshape(8)
    cpackf[:, 24:152] = np.eye(128, dtype=np.float32)

    shared = dict(cpackr=cpackr, cpackf=cpackf)
    in_maps = []
    for c in range(N_CORES):
        m = dict(shared)
        m["x"] = np.ascontiguousarray(x[c * BPC:(c + 1) * BPC])
        m["g"] = np.ascontiguousarray(g[c * BPC:(c + 1) * BPC])
        in_maps.append(m)
    return in_maps


_NC = None


def kernel(x, mask_weight, ln_weight, ln_bias, gumbel_noise, init_flag=None,
           **_ignored):
    global _NC
    from concourse.bass_utils import run_bass_kernel_spmd

    if _NC is None:
        _NC = build_nc()
    in_maps = host_inputs(x, mask_weight, ln_weight, ln_bias, gumbel_noise)
    res = run_bass_kernel_spmd(_NC, in_maps, list(range(N_CORES))).results

    # all 512 output channels are identical: broadcast the per-image map
    out = np.empty((B, C, H, W), dtype=np.float32)
    for c in range(N_CORES):
        maps = res[c]["out"].reshape(BPC, H, W)
        out[c * BPC:(c + 1) * BPC] = maps[:, None, :, :]
    return out


# revision 22
# speedup vs baseline: 1.0461x; 1.0461x over previous
"""Trainium2 Bass kernel for nn_Conv2d_selfAdapt (dense_cnn).

Math reduction (derived from the reference):
  The final einsum weight[(c*9+p), j] = KERN[p] is independent of output
  channel j, so all 512 output channels are identical:
      out[b, :, h, w] = S[b,h,w] - sum_p mask_p[b,h,w] * Sshift_p[b,h,w]
  where S = channel-sum of x, Sshift_p = zero-padded spatial shift of S,
  and mask = straight-through one-hot of argmax over the 8 gate channels
  (softmax is monotone, theta=1 -> argmax(LN(conv(x,w)) + gumbel)).

  The only heavy compute is the 3x3 conv (512 -> 8 ch).  It is computed as
  a 1x1 conv with M=81 outputs (9 taps x 8 ch + a ones-row giving S, taps
  duplicated) in fp32r (full PE rate), then the 9 per-tap partial maps are
  spatially shifted (zero-padded SBUF grid + shifted-window SBUF DMAs) and
  summed with a K=81 selection matmul (fp32r).  Per-pixel LN/gumbel/argmax/
  select runs on the vector engine in a pixel-partition layout obtained
  with 4 wide PE transposes per image (exact fp32 to protect the argmax).

  The device emits only the single [H*W] map per image; the host
  broadcasts it to the 512 identical output channels.

Sharding: pure data parallel, 2 images per core across 8 cores.
"""

import os
import sys

import numpy as np

for _p in ("/opt/trn_rl_repo", "/root/.axon_site/_ro/trn_rl_repo"):
    if os.path.isdir(_p) and _p not in sys.path:
        sys.path.insert(0, _p)

import concourse.bass as bass
import concourse.bacc as bacc
import concourse.tile as tile
from concourse import mybir
from contextlib import ExitStack

B, C, H, W = 16, 512, 48, 48
N_CORES = 8
BPC = B // N_CORES          # images per core
HW = H * W                  # 2304
G = W + 2                   # padded grid side (50)
EPS_LN = 1e-6
BIG = 1000.0
FP = mybir.dt.float32
FR = mybir.dt.float32r
# conv/sel chunks: 48-col row-aligned so PSUM copies stay rectangular
CCHUNKS = [(0, 10), (10, 10), (20, 10), (30, 10), (40, 8)]  # (row0, nrows)
# pack chunks: 128-px block aligned (4x512 + 256) for the transposes
PCHUNKS = [(0, 512), (512, 512), (1024, 512), (1536, 512), (2048, 256)]
NBLK = HW // 128            # 18 valid pixel blocks per image (20 padded)
# flat padded conv-partial layout: row pitch 49 (48 data + 1 zero spacer),
# data cell (r, w) at 50 + 49*r + w; head [0:50] and tail [2402:2452] zero.
GP = 49
GBASE = 50
GTOT = GBASE + GP * 48 + GP + 1  # 2452
PSHW = GP * 48               # 2352 (shifted window length per tap)

AL = mybir.AluOpType
AX = mybir.AxisListType


def build_nc():
    nc = bacc.Bacc("TRN2", target_bir_lowering=False, debug=False,
                   num_devices=N_CORES)

    x_d = nc.dram_tensor("x", [BPC, C, HW], FR, kind="ExternalInput")
    g_d = nc.dram_tensor("g", [BPC, 8, HW], FP, kind="ExternalInput")
    # merged constants: one FR pack (w73 | sel) + one FP pack
    # (iota | iotab | lnw | ident) so startup needs only two const DMAs
    cr_d = nc.dram_tensor("cpackr", [128, 341], FR, kind="ExternalInput")
    cf_d = nc.dram_tensor("cpackf", [128, 152], FP, kind="ExternalInput")
    out_d = nc.dram_tensor("out", [BPC, HW], FP, kind="ExternalOutput")

    with tile.TileContext(nc) as tc, ExitStack() as ctx:
        consts = ctx.enter_context(tc.tile_pool(name="consts", bufs=1))
        xpool = ctx.enter_context(tc.tile_pool(name="xp", bufs=2))
        work = ctx.enter_context(tc.tile_pool(name="work", bufs=2))
        vp = ctx.enter_context(tc.tile_pool(name="vp", bufs=2))
        psA = ctx.enter_context(tc.tile_pool(name="psA", bufs=3, space="PSUM"))
        psS = ctx.enter_context(tc.tile_pool(name="psS", bufs=2, space="PSUM"))
        psT = ctx.enter_context(tc.tile_pool(name="psT", bufs=1, space="PSUM"))
        psO = ctx.enter_context(tc.tile_pool(name="psO", bufs=1, space="PSUM"))

        cr = consts.tile([128, 341], FR, tag="cr")
        cf = consts.tile([128, 152], FP, tag="cf")
        eps_t = consts.tile([128, 1], FP, tag="eps")
        nc.vector.memset(eps_t, EPS_LN)
        nc.gpsimd.dma_start(out=cr, in_=cr_d[:])
        nc.gpsimd.dma_start(out=cf, in_=cf_d[:])
        w73 = cr[:, 0:324].rearrange("p (kt m) -> p kt m", m=81)
        sel = cr[0:81, 324:341]
        iota = cf[:, 0:8]
        iotab = cf[:, 8:16]
        lnw = cf[:, 16:24]
        ident = cf[:, 24:152]
        # warm the Sqrt activation table before the timed pipeline needs it
        warm_t = consts.tile([128, 1], FP, tag="warm")
        nc.scalar.activation(warm_t, eps_t, mybir.ActivationFunctionType.Sqrt,
                             bias=eps_t, scale=1.0)

        lnw_b = lnw.unsqueeze(1).broadcast_to([128, 20, 8])
        iota_b = iota.unsqueeze(1).broadcast_to([128, 20, 8])
        iotab_b = iotab.unsqueeze(1).broadcast_to([128, 20, 8])

        # ---- input loads (half-tile DMAs, image 0 first) -------------------
        # DMA issue costs ~5ns/descriptor, so descriptors must be full
        # 9216B rows: [128, 1152] halves are 128 descriptors each.  All of
        # x goes on the sync HWDGE ring in image order; the ~9 global DMA
        # completion lanes throttle image 1 behind image 0 naturally.
        # Shift DMAs use the scalar ring, small stuff the gpsimd ring.
        xt = []
        for b in range(BPC):
            row = []
            for kt in range(4):
                t = xpool.tile([128, HW], FR, tag=f"x{kt}")
                row.append(t)
            xt.append(row)

        # packed layout: 5 row-groups of 25 rows (17 sel outputs + 8 gumbel);
        # row-group rg covers pixels [512*rg, 512*rg+512)
        packs = []
        for b in range(BPC):
            packed = work.tile([125, 512], FP, tag="packed")
            packs.append(packed)
            # pixel blocks 18,19 don't exist; zero them so the vector
            # pipeline sees benign values in the padded lanes (rows 96:100
            # are re-covered by the rg=3 gumbel DMA afterwards)
            nc.scalar.memzero(packed[96:125, 256:512])
            for rg, (off, ncol) in enumerate(PCHUNKS):
                nc.gpsimd.dma_start(
                    out=packed[25 * rg + 17:25 * rg + 25, 0:ncol],
                    in_=g_d[b, :, off:off + ncol])

        for b in range(BPC):
            for h in range(2):
                for kt in range(4):
                    nc.sync.dma_start(
                        out=xt[b][kt][:, h * 1152:(h + 1) * 1152],
                        in_=x_d[b, kt * 128:(kt + 1) * 128,
                                h * 1152:(h + 1) * 1152])

        for b in range(BPC):
            # ---- conv as 1x1 matmul in fp32r, row-aligned chunks ----------
            # gridF: flat zero-padded layout, row pitch 49 (spacer column
            # absorbs horizontal shift wrap; head/tail absorb vertical)
            gridF = work.tile([81, GTOT], FR, tag="grid")
            nc.scalar.memzero(gridF[:, 0:GBASE])
            nc.scalar.memzero(gridF[:, GBASE + GP * 48:GTOT])
            nc.scalar.memzero(
                gridF[:, GBASE:GBASE + GP * 48].rearrange(
                    "p (r w) -> p r w", w=GP)[:, :, 48:49])
            for grp in range(0, len(CCHUNKS), 2):
                pair = CCHUNKS[grp:grp + 2]
                Ps = []
                for _pi in range(len(pair)):
                    Pt = psA.tile([81, 480], FP, tag="A")
                    Ps.append(Pt)
                for kt in range(4):
                    # adjacent matmuls share the stationary weights
                    for P, (r0, nr) in zip(Ps, pair):
                        nc.tensor.matmul(
                            P[:, 0:nr * 48],
                            w73[:, kt, :],
                            xt[b][kt][:, r0 * 48:(r0 + nr) * 48],
                            start=(kt == 0),
                            stop=(kt == 3),
                        )
                for P, (r0, nr) in zip(Ps, pair):
                    nc.scalar.copy(
                        out=gridF[:, GBASE + GP * r0:GBASE + GP * (r0 + nr)]
                        .rearrange("p (r w) -> p r w", w=GP)[:, :, 0:48],
                        in_=P[:, 0:nr * 48].rearrange("p (h w) -> p h w", w=W),
                    )

            # ---- shifted per-tap rows: one contiguous run per partition --
            psh = work.tile([81, PSHW], FR, tag="psh")
            for t in range(9):
                ti, tj = divmod(t, 3)
                s = GBASE + GP * (ti - 1) + (tj - 1)
                nc.scalar.dma_start(
                    out=psh[t * 9:(t + 1) * 9, :],
                    in_=gridF[t * 9:(t + 1) * 9, s:s + PSHW],
                )
            pshv = psh.rearrange("p (r w) -> p r w", w=GP)

            # ---- selection matmul (fp32r): rows 0-7 logits, 8+t = Sp_t ----
            # compute engines can only start at partition 0/32/64/96, so
            # results go PSUM -> whole-image stage -> (DMA) packed row 25*rg
            packed = packs[b]
            stage = work.tile([17, HW], FP, tag="stage")
            for (r0, nr) in CCHUNKS:
                ncol = nr * 48
                Lc = psS.tile([17, 480], FP, tag="S")
                nc.tensor.matmul(Lc[:, 0:ncol], sel,
                                 pshv[:, r0:r0 + nr, 0:48],
                                 start=True, stop=True)
                nc.vector.tensor_copy(stage[:, r0 * 48:r0 * 48 + ncol],
                                      Lc[:, 0:ncol])
            for rg, (off, ncol) in enumerate(PCHUNKS):
                nc.gpsimd.dma_start(
                    out=packed[25 * rg:25 * rg + 17, 0:ncol],
                    in_=stage[:, off:off + ncol])

            # ---- transpose to pixel-partition layout (exact fp32) ---------
            # tt free layout [rg, cg, 25] so block index k = 4*rg+cg equals
            # the pixel-block index -> output rows come out in order
            tt = work.tile([128, 5, 4, 25], FP, tag="tt")
            for cg in range(4):
                tp = psT.tile([128, 125], FP, tag="T")
                nc.tensor.transpose(tp,
                                    packed[:, cg * 128:(cg + 1) * 128],
                                    ident[0:125, 0:125])
                nc.vector.tensor_copy(
                    tt[:, :, cg, :],
                    tp.rearrange("p (rg j) -> p rg j", j=25))
            ttv = tt.rearrange("p rg cg j -> p (rg cg) j")

            # ---- per-pixel LN + gumbel + argmax + neighbor select ---------
            Lap = ttv[:, :, 0:8]
            mu = vp.tile([128, 20], FP, tag="mu")
            nc.vector.tensor_reduce(mu, Lap, axis=AX.X, op=AL.add)
            cen = vp.tile([128, 20, 8], FP, tag="cen")
            nc.vector.scalar_tensor_tensor(
                cen, in0=mu.unsqueeze(2).broadcast_to([128, 20, 8]),
                scalar=-1.0 / 8.0, in1=Lap, op0=AL.mult, op1=AL.add)
            sq = vp.tile([128, 20, 8], FP, tag="sq")
            nc.vector.tensor_tensor(sq, cen, cen, op=AL.mult)
            v8 = vp.tile([128, 20], FP, tag="v8")
            nc.vector.tensor_reduce(v8, sq, axis=AX.X, op=AL.add)
            sd = vp.tile([128, 20], FP, tag="sd")
            nc.scalar.activation(sd, v8, mybir.ActivationFunctionType.Sqrt,
                                 bias=eps_t, scale=1.0 / 8.0)
            rstd = vp.tile([128, 20], FP, tag="rstd")
            nc.vector.reciprocal(rstd, sd)
            rl = vp.tile([128, 20, 8], FP, tag="rl")
            nc.vector.tensor_tensor(
                rl, rstd.unsqueeze(2).broadcast_to([128, 20, 8]), lnw_b,
                op=AL.mult)
            z = vp.tile([128, 20, 8], FP, tag="z")
            nc.vector.tensor_tensor(z, cen, rl, op=AL.mult)
            # add gumbel (+ln_bias, folded on host)
            z2 = vp.tile([128, 20, 8], FP, tag="z2")
            nc.vector.tensor_tensor(z2, z, ttv[:, :, 17:25], op=AL.add)

            mx = vp.tile([128, 20], FP, tag="mx")
            nc.vector.tensor_reduce(mx, z2, axis=AX.X, op=AL.max)
            eq = vp.tile([128, 20, 8], FP, tag="eq")
            nc.vector.tensor_tensor(
                eq, z2, mx.unsqueeze(2).broadcast_to([128, 20, 8]),
                op=AL.is_equal)
            im = vp.tile([128, 20, 8], FP, tag="im")
            nc.vector.scalar_tensor_tensor(
                im, in0=eq, scalar=-BIG, in1=iotab_b, op0=AL.mult, op1=AL.add)
            am = vp.tile([128, 20], FP, tag="am")
            nc.vector.tensor_reduce(am, im, axis=AX.X, op=AL.min)
            hard = vp.tile([128, 20, 8], FP, tag="hard")
            nc.vector.tensor_tensor(
                hard, iota_b, am.unsqueeze(2).broadcast_to([128, 20, 8]),
                op=AL.is_equal)

            # pair one-hot lanes with the 8 non-center taps (skip center=12)
            prod = vp.tile([128, 20, 8], FP, tag="prod")
            nc.vector.tensor_tensor(prod[:, :, 0:4], hard[:, :, 0:4],
                                    ttv[:, :, 8:12], op=AL.mult)
            nc.vector.tensor_tensor(prod[:, :, 4:8], hard[:, :, 4:8],
                                    ttv[:, :, 13:17], op=AL.mult)
            selS = vp.tile([128, 20], FP, tag="selS")
            nc.vector.tensor_reduce(selS, prod, axis=AX.X, op=AL.add)
            outm = vp.tile([128, 20], FP, tag="outm")
            nc.vector.tensor_tensor(outm, ttv[:, :, 12], selS,
                                    op=AL.subtract)

            # ---- transpose map back to pixel order and store --------------
            po = psO.tile([20, 128], FP, tag="O")
            nc.tensor.transpose(po, outm, ident)
            outsb = vp.tile([20, 128], FP, tag="outsb")
            nc.vector.tensor_copy(outsb, po)
            nc.gpsimd.dma_start(
                out=out_d[b, :].rearrange("(j f) -> j f", f=128),
                in_=outsb[0:18, :])

    nc.compile()
    return nc


def _to_fp32r(a):
    """Round fp32 -> fp32r (11-bit mantissa, low 12 bits zero, RNE).

    The PE's fp32r datapath reads only the top 20 bits; pre-rounding on
    the host lets the kernel DMA the data straight into float32r tiles.
    """
    u = np.ascontiguousarray(a, dtype=np.float32).view(np.uint32)
    r = (u + 0x7FF + ((u >> 12) & 1)) & np.uint32(0xFFFFF000)
    return r.view(np.float32)


def host_inputs(x, mask_weight, ln_weight, ln_bias, gumbel_noise):
    """Build per-core input maps (numpy only)."""
    x = _to_fp32r(np.asarray(x, dtype=np.float32)).reshape(B, C, HW)
    g = np.asarray(gumbel_noise, dtype=np.float32).reshape(B, 8, HW)
    # fold the LN bias into the (precomputed) gumbel noise
    g = np.ascontiguousarray(
        g + np.asarray(ln_bias, np.float32).reshape(1, 8, 1))

    mw = np.asarray(mask_weight, dtype=np.float32).reshape(8, C, 9)
    a = mw.transpose(1, 2, 0)                         # [c, tap, o]
    w73 = np.ones((C, 9, 9), dtype=np.float32)        # [c, tap, o|ones]
    w73[:, :, :8] = a
    w73 = w73.reshape(4, 128, 81).transpose(1, 0, 2)  # [c_mod, kt, m]
    w73 = _to_fp32r(w73)

    sel = np.zeros((81, 17), dtype=np.float32)
    for t in range(9):
        for o in range(8):
            sel[t * 9 + o, o] = 1.0
        sel[t * 9 + 8, 8 + t] = 1.0
    cpackr = np.zeros((128, 341), dtype=np.float32)
    cpackr[:, 0:324] = w73.reshape(128, 324)
    cpackr[0:81, 324:341] = sel
    cpackf = np.zeros((128, 152), dtype=np.float32)
    cpackf[:, 0:8] = np.arange(8, dtype=np.float32)
    cpackf[:, 8:16] = np.arange(8, dtype=np.float32) + BIG
    cpackf[:, 16:24] = np.asarray(ln_weight, np.float32).reshape(8)
    cpackf[:, 24:152] = np.eye(128, dtype=np.float32)

    shared = dict(cpackr=cpackr, cpackf=cpackf)
    in_maps = []
    for c in range(N_CORES):
        m = dict(shared)
        m["x"] = np.ascontiguousarray(x[c * BPC:(c + 1) * BPC])
        m["g"] = np.ascontiguousarray(g[c * BPC:(c + 1) * BPC])
        in_maps.append(m)
    return in_maps


_NC = None


def kernel(x, mask_weight, ln_weight, ln_bias, gumbel_noise, init_flag=None,
           **_ignored):
    global _NC
    from concourse.bass_utils import run_bass_kernel_spmd

    if _NC is None:
        _NC = build_nc()
    in_maps = host_inputs(x, mask_weight, ln_weight, ln_bias, gumbel_noise)
    res = run_bass_kernel_spmd(_NC, in_maps, list(range(N_CORES))).results

    # all 512 output channels are identical: broadcast the per-image map
    out = np.empty((B, C, H, W), dtype=np.float32)
    for c in range(N_CORES):
        maps = res[c]["out"].reshape(BPC, H, W)
        out[c * BPC:(c + 1) * BPC] = maps[:, None, :, :]
    return out
